# revision 10
# baseline (speedup 1.0000x reference)
"""Trainium2 Bass kernel for nn_PhysicsLoss.

loss = mean|pred - target|
     + 0.1 * mean_b|soft_argmax_win(pred) - soft_argmax_win(target)| / T

where the window is [c-20, c+20) around c = argmax|target| per row, and
soft_argmax uses softmax(25*|x|) restricted to the window.

Sharding: pure data parallel, 8 cores x 512 rows. Each core streams its
[512, 6000] shards once (memory-bound), chunked [128,2000] DMAs on two HWDGE
rings (pred via sync, target via scalar). Work is balanced across engines:
DVE does the subtract + 1/3 of the blocked abs-max, GpSimd does 2/3 of the
blocked abs-max + window gathers + offset math, ACT does the |.| + row-sum
accumulation and the softmax exps. The argmax block is found with the top-8
max/max_index DVE ops, a 120-element window (3 blocks, always covering
[c-20, c+20)) is gathered from DRAM with a per-partition indirect DMA, and
masked softmax picks use window-local positions (the window offset cancels
in |pick_pred - pick_true|). Host combines the tiny per-core partials in f64.
"""

import sys

if "/opt/trn_rl_repo" not in sys.path:
    sys.path.insert(0, "/opt/trn_rl_repo")

from contextlib import ExitStack

import numpy as np

import concourse.bass as bass
import concourse.tile as tile
from concourse import bacc, mybir
from concourse._compat import with_exitstack
from concourse.bass_utils import run_bass_kernel_spmd

B, T = 4096, 6000
N_CORES = 8
ROWS = B // N_CORES      # 512 rows per core
P = 128                  # partitions
N_TILES = ROWS // P      # 4 row-tiles per core
CHUNK = 2000
N_CHUNKS = T // CHUNK    # 3
BLK = 40                 # block width; window [c-20, c+20) spans <= 3 blocks
BLOCKS = T // BLK        # 150
BPC = CHUNK // BLK       # 50 blocks per chunk
WIN = 3 * BLK            # 120-wide gathered window
PICK_WIN = 20
BETA = 25.0
W_PICK = 0.1

# consts table column layout
C_IOTA120 = 0            # [0,120): iota over window (local positions)
C_RB = WIN               # [120,124): col t = (t*128 + p) * 6000 (row base)
C_TOT = WIN + N_TILES    # 124

F32 = mybir.dt.float32
ALU = mybir.AluOpType
ACTF = mybir.ActivationFunctionType
AXX = mybir.AxisListType.X


def _build_consts() -> np.ndarray:
    c = np.zeros((P, C_TOT), np.float32)
    c[:, C_IOTA120:C_IOTA120 + WIN] = np.arange(WIN)[None, :]
    for t in range(N_TILES):
        c[:, C_RB + t] = (t * P + np.arange(P)) * T
    return c


@with_exitstack
def _phys_loss_kernel(ctx: ExitStack, tc: tile.TileContext,
                      pred: bass.AP, target: bass.AP,
                      consts: bass.AP, out: bass.AP):
    nc = tc.nc

    cpool = ctx.enter_context(tc.tile_pool(name="cpool", bufs=1))
    ppool = ctx.enter_context(tc.tile_pool(name="ppool", bufs=4))
    tpool = ctx.enter_context(tc.tile_pool(name="tpool", bufs=4))
    dpool = ctx.enter_context(tc.tile_pool(name="dpool", bufs=2))
    spool = ctx.enter_context(tc.tile_pool(name="spool", bufs=2))

    ct = cpool.tile([P, C_TOT], F32)
    nc.gpsimd.dma_start(ct[:], consts[:, :])
    iota120 = ct[:, C_IOTA120:C_IOTA120 + WIN]

    outsb = cpool.tile([P, 3 * N_TILES], F32, tag="outsb")

    for t in range(N_TILES):
        r0 = t * P
        pcs, tcs = [], []
        for ci in range(N_CHUNKS):
            c0 = ci * CHUNK
            pc = ppool.tile([P, CHUNK], F32, tag="pc")
            nc.sync.dma_start(pc[:], pred[r0:r0 + P, c0:c0 + CHUNK])
            tcn = tpool.tile([P, CHUNK], F32, tag="tc")
            nc.scalar.dma_start(tcn[:], target[r0:r0 + P, c0:c0 + CHUNK])
            pcs.append(pc)
            tcs.append(tcn)

        bmax = spool.tile([P, BLOCKS], F32, tag="bmax")
        s1c = spool.tile([P, N_CHUNKS], F32, tag="s1c")
        for ci in range(N_CHUNKS):
            d = dpool.tile([P, CHUNK], F32, tag="d")
            nc.vector.tensor_tensor(out=d[:], in0=pcs[ci][:], in1=tcs[ci][:],
                                    op=ALU.subtract)
            ad = dpool.tile([P, CHUNK], F32, tag="ad")
            nc.scalar.activation(out=ad[:], in_=d[:], func=ACTF.Abs,
                                 accum_out=s1c[:, ci:ci + 1])
            nc.vector.tensor_reduce(
                out=bmax[:, ci * BPC:(ci + 1) * BPC],
                in_=tcs[ci][:].rearrange("p (b w) -> p b w", w=BLK),
                axis=AXX, op=ALU.max, apply_absolute_value=True)
        nc.vector.tensor_reduce(out=outsb[:, 3 * t:3 * t + 1], in_=s1c[:],
                                axis=AXX, op=ALU.add)

        # ---- phase 2: windowed soft-argmax picks ----
        # top-1 block: m = max|t| and its (first) block index
        mx8 = spool.tile([P, 8], F32, tag="mx8")
        mi8 = spool.tile([P, 8], mybir.dt.uint32, tag="mi8")
        nc.vector.max(mx8[:], bmax[:])
        nc.vector.max_index(mi8[:], mx8[:], bmax[:])
        m = mx8[:, 0:1]
        bstar = mi8[:, 0:1]

        # gather start gs40 = clamp(b*-1, 0, 147) * 40, flat DRAM offsets
        g0 = spool.tile([P, 1], F32, tag="g0")
        nc.gpsimd.tensor_scalar(out=g0[:], in0=bstar, scalar1=1.0,
                                scalar2=0.0, op0=ALU.subtract, op1=ALU.max)
        gs40 = spool.tile([P, 1], F32, tag="gs40")
        nc.gpsimd.tensor_scalar(out=gs40[:], in0=g0[:],
                                scalar1=float(BLOCKS - 3), scalar2=float(BLK),
                                op0=ALU.min, op1=ALU.mult)
        offs_f = spool.tile([P, 1], F32, tag="offs_f")
        nc.gpsimd.tensor_scalar(out=offs_f[:], in0=ct[:, C_RB + t:C_RB + t + 1],
                                scalar1=gs40[:], op0=ALU.add, scalar2=None)
        offs_i = spool.tile([P, 1], mybir.dt.int32, tag="offs_i")
        nc.gpsimd.tensor_copy(out=offs_i[:], in_=offs_f[:])

        tw = spool.tile([P, WIN], F32, tag="tw")
        nc.gpsimd.indirect_dma_start(
            out=tw[:], out_offset=None, in_=target[:, :],
            in_offset=bass.IndirectOffsetOnAxis(ap=offs_i[:], axis=1))
        pw = spool.tile([P, WIN], F32, tag="pw")
        nc.gpsimd.indirect_dma_start(
            out=pw[:], out_offset=None, in_=pred[:, :],
            in_offset=bass.IndirectOffsetOnAxis(ap=offs_i[:], axis=1))

        atw = spool.tile([P, WIN], F32, tag="atw")
        nc.scalar.activation(out=atw[:], in_=tw[:], func=ACTF.Abs)
        apw = spool.tile([P, WIN], F32, tag="apw")
        nc.scalar.activation(out=apw[:], in_=pw[:], func=ACTF.Abs)

        # exact (first) argmax position within the window, local coords
        w8 = spool.tile([P, 8], F32, tag="w8")
        cl8 = spool.tile([P, 8], mybir.dt.uint32, tag="cl8")
        nc.vector.max(w8[:], atw[:])
        nc.vector.max_index(cl8[:], w8[:], atw[:])
        cl = cl8[:, 0:1]

        # mask over local positions: [cl-20, cl+20) (global clamps coincide)
        lo = spool.tile([P, 1], F32, tag="lo")
        nc.vector.tensor_scalar(out=lo[:], in0=cl, scalar1=float(PICK_WIN),
                                op0=ALU.subtract, scalar2=None)
        hi = spool.tile([P, 1], F32, tag="hi")
        nc.vector.tensor_scalar(out=hi[:], in0=cl, scalar1=float(PICK_WIN),
                                op0=ALU.add, scalar2=None)
        m1 = spool.tile([P, WIN], F32, tag="m1")
        nc.vector.tensor_scalar(out=m1[:], in0=iota120, scalar1=lo[:],
                                op0=ALU.is_ge, scalar2=None)
        mask = spool.tile([P, WIN], F32, tag="mask")
        nc.vector.scalar_tensor_tensor(out=mask[:], in0=iota120, scalar=hi[:],
                                       in1=m1[:], op0=ALU.is_lt, op1=ALU.mult)

        # target softmax: masked max is m exactly (argmax inside mask);
        # masked-out exp(-25m) underflows to ~0, so no re-mask needed.
        amt = spool.tile([P, WIN], F32, tag="amt")
        nc.vector.tensor_tensor(out=amt[:], in0=atw[:], in1=mask[:], op=ALU.mult)
        negm = spool.tile([P, 1], F32, tag="negm")
        nc.vector.tensor_scalar(out=negm[:], in0=m, scalar1=-BETA,
                                op0=ALU.mult, scalar2=None)
        et = spool.tile([P, WIN], F32, tag="et")
        st = spool.tile([P, 1], F32, tag="st")
        nc.scalar.activation(out=et[:], in_=amt[:], func=ACTF.Exp,
                             scale=BETA, bias=negm[:], accum_out=st[:])
        wdt = spool.tile([P, WIN], F32, tag="wdt")
        wt = spool.tile([P, 1], F32, tag="wt")
        nc.vector.scalar_tensor_tensor(out=wdt[:], in0=et[:], scalar=1.0,
                                       in1=iota120, op0=ALU.mult, op1=ALU.mult,
                                       accum_out=wt[:])
        rst = spool.tile([P, 1], F32, tag="rst")
        nc.vector.reciprocal(rst[:], st[:])
        nc.vector.tensor_tensor(out=outsb[:, 3 * t + 2:3 * t + 3], in0=wt[:],
                                in1=rst[:], op=ALU.mult)

        # pred softmax over the same mask
        amp = spool.tile([P, WIN], F32, tag="amp")
        nc.vector.tensor_tensor(out=amp[:], in0=apw[:], in1=mask[:], op=ALU.mult)
        mp = spool.tile([P, 1], F32, tag="mp")
        nc.vector.tensor_reduce(out=mp[:], in_=amp[:], axis=AXX, op=ALU.max)
        negmp = spool.tile([P, 1], F32, tag="negmp")
        nc.vector.tensor_scalar(out=negmp[:], in0=mp[:], scalar1=-BETA,
                                op0=ALU.mult, scalar2=None)
        ep = spool.tile([P, WIN], F32, tag="ep")
        sp = spool.tile([P, 1], F32, tag="sp")
        nc.scalar.activation(out=ep[:], in_=amp[:], func=ACTF.Exp,
                             scale=BETA, bias=negmp[:], accum_out=sp[:])
        wdp = spool.tile([P, WIN], F32, tag="wdp")
        wp = spool.tile([P, 1], F32, tag="wp")
        nc.vector.scalar_tensor_tensor(out=wdp[:], in0=ep[:], scalar=1.0,
                                       in1=iota120, op0=ALU.mult, op1=ALU.mult,
                                       accum_out=wp[:])
        rsp = spool.tile([P, 1], F32, tag="rsp")
        nc.vector.reciprocal(rsp[:], sp[:])
        nc.vector.tensor_tensor(out=outsb[:, 3 * t + 1:3 * t + 2], in0=wp[:],
                                in1=rsp[:], op=ALU.mult)

    nc.gpsimd.dma_start(out[:, :], outsb[:])


_COMPILED = None


def _get_compiled():
    global _COMPILED
    if _COMPILED is None:
        nc = bacc.Bacc("TRN2", target_bir_lowering=False, debug=False)
        pred = nc.dram_tensor("pred", [ROWS, T], F32, kind="ExternalInput").ap()
        target = nc.dram_tensor("target", [ROWS, T], F32, kind="ExternalInput").ap()
        consts = nc.dram_tensor("consts", [P, C_TOT], F32, kind="ExternalInput").ap()
        out = nc.dram_tensor("out", [P, 3 * N_TILES], F32, kind="ExternalOutput").ap()
        with tile.TileContext(nc) as tc:
            _phys_loss_kernel(tc, pred, target, consts, out)
        nc.compile()
        _COMPILED = nc
    return _COMPILED


def _run(pred: np.ndarray, target: np.ndarray, trace: bool = False):
    nc = _get_compiled()
    consts = _build_consts()
    pred = np.ascontiguousarray(pred, dtype=np.float32)
    target = np.ascontiguousarray(target, dtype=np.float32)
    in_maps = [
        {
            "pred": pred[k * ROWS:(k + 1) * ROWS],
            "target": target[k * ROWS:(k + 1) * ROWS],
            "consts": consts,
        }
        for k in range(N_CORES)
    ]
    res = run_bass_kernel_spmd(nc, in_maps, list(range(N_CORES)), trace=trace)
    s1 = 0.0
    perr = 0.0
    for k in range(N_CORES):
        o = res.results[k]["out"].astype(np.float64)
        s1 += o[:, 0::3].sum()
        perr += np.abs(o[:, 1::3] - o[:, 2::3]).sum()
    loss = s1 / (B * T)
    pick = (perr / B) / T
    total = np.array(loss + W_PICK * pick, dtype=np.float32)
    return total, res


def kernel(pred: np.ndarray, target: np.ndarray) -> np.ndarray:
    total, _ = _run(pred, target, trace=False)
    return total
